# revision 1
# baseline (speedup 1.0000x reference)
"""Trainium2 Bass kernel for nn_MultiHeadAttn (unnormalized spatial attention).

Reference computation (per sample s of B=16):
    X = a[s]               # [C=256, HW=4096]  (H=64 rows of W=64)
    QT = wq @ X + bq       # [C, HW]   (q channels on rows)
    KT = wk @ X + bk
    V  = (wv @ X + bv).T   # [HW, C]   (hw on rows)
    per h: attnT_h = K_h @ Q_h^T        # [W, W]  == (Q_h K_h^T)^T
           attoutT_h = V_h^T @ attnT_h  # [C, W]
    out[s] = a[s] + attoutT (reassembled [C, HW])

Sharding: data-parallel over batch, 2 samples per core on 8 cores.
All matmuls in bf16 (fp32 PSUM accumulation); residual added in fp32.
"""

import numpy as np
import ml_dtypes

import concourse.bass as bass
import concourse.mybir as mybir
import concourse.tile as tile
from concourse import bacc
from concourse.bass_utils import run_bass_kernel_spmd

BF16 = mybir.dt.bfloat16
F32 = mybir.dt.float32
AF = mybir.ActivationFunctionType

N_CORES = 8
B, C, H, W = 16, 256, 64, 64
HW = H * W               # 4096
S = B // N_CORES         # samples per core = 2
CC = C // 128            # channel chunks = 2


def build_program(repeat=1):
    # repeat > 1 re-processes the same samples (timing amplification only)
    nc = bacc.Bacc("TRN2", target_bir_lowering=False, debug=False)

    a_in = nc.dram_tensor("a_bf", [S, C, HW], BF16, kind="ExternalInput")
    # packed constants (see _make_in_maps): weights [128, 3*512] bf16 with
    # w_all[p, w*512 + cc*256 + o] = w^T[cc*128 + p, o]; biases [128, 4] f32
    # as columns (bq0, bq1, bk0, bk1); bvb [128, 512] f32 = bv tiled twice.
    w_in = nc.dram_tensor("w_all", [128, 3 * 512], BF16, kind="ExternalInput")
    bqk_in = nc.dram_tensor("bqk", [128, 4], F32, kind="ExternalInput")
    bvb_in = nc.dram_tensor("bvb", [128, 512], F32, kind="ExternalInput")
    out_d = nc.dram_tensor("out", [S, C, HW], F32, kind="ExternalOutput")

    with tile.TileContext(nc) as tc:
        with (
            tc.tile_pool(name="const", bufs=1) as const_pool,
            tc.tile_pool(name="xb", bufs=4) as xb_pool,
            tc.tile_pool(name="qk", bufs=2) as qk_pool,
            tc.tile_pool(name="vsb", bufs=2) as v_pool,
            tc.tile_pool(name="atsb", bufs=6) as at_pool,
            tc.tile_pool(name="osb", bufs=8) as out_pool,
            tc.tile_pool(name="pp", bufs=4, space=bass.MemorySpace.PSUM) as pp_pool,
            tc.tile_pool(name="atp", bufs=2, space=bass.MemorySpace.PSUM) as atp_pool,
            tc.tile_pool(name="aop", bufs=2, space=bass.MemorySpace.PSUM) as aop_pool,
        ):
            # --- input load helper ---
            # a is pre-cast to bf16 on the host; eighth-blocks (interleaved
            # across the two channel chunks) DMA straight into the xb tiles
            # so projections can start on early columns.
            QB = HW // 8
            xb_all = {sv: [xb_pool.tile([128, HW], BF16, tag="xb", name="xb")
                           for _ in range(CC)] for sv in range(S * repeat)}

            def emit_load(sv, q8, eng=None):
                s = sv % S
                # bulk loads ride the software-DGE queue (Q7-armed) so the
                # SP hardware ring is free for outputs; the first few blocks
                # use the SP ring, which starts up faster
                eng = eng if eng is not None else nc.sync
                for cc in range(CC):
                    eng.dma_start(
                        xb_all[sv][cc][:, q8 * QB:(q8 + 1) * QB],
                        a_in[s, cc * 128:(cc + 1) * 128,
                             q8 * QB:(q8 + 1) * QB])

            # --- persistent constants ---
            # weight view layout: [128 part = c_in within chunk, cols =
            # cc*256 + c_out] per projection.  Only wq + biases are loaded
            # ahead of the first input block; wk/wv follow it so the DMA
            # ring delivers compute-critical bytes first.
            w_all_sb = const_pool.tile([128, 3 * 512], BF16, tag="w_all")
            bqk_sb = const_pool.tile([128, 4], F32, tag="bqk")
            bvb_sb = const_pool.tile([128, 512], F32, tag="bvb")
            nc.sync.dma_start(w_all_sb[:, 0:512], w_in[:, 0:512])
            nc.sync.dma_start(bqk_sb[:], bqk_in[:])
            emit_load(0, 0, eng=nc.sync)
            nc.sync.dma_start(w_all_sb[:, 512:1024], w_in[:, 512:1024])
            nc.sync.dma_start(w_all_sb[:, 1024:1536], w_in[:, 1024:1536])
            nc.sync.dma_start(bvb_sb[:], bvb_in[:])
            w_sb = {"wq": w_all_sb[:, 0:512],
                    "wk": w_all_sb[:, 512:1024],
                    "wv": w_all_sb[:, 1024:1536]}
            bq_sb = bqk_sb[:, 0:2]
            bk_sb = bqk_sb[:, 2:4]

            emit_load(0, 1, eng=nc.sync)
            for q8 in range(2, 8):
                emit_load(0, q8)

            for sv in range(S * repeat):
                s = sv % S
                if sv > 0:
                    for q8 in range(8):
                        emit_load(sv, q8)
                xb = xb_all[sv]

                # --- fused per-column-block pipeline ---
                # For each of 8 column blocks (512 hw positions = 8 h rows):
                # Q/K/V projections for that block, then attention for the
                # same block.  This spreads PE/ACT/DVE work evenly in time.
                qt = [qk_pool.tile([128, HW], BF16, tag=f"qt{oc}", name=f"qt{oc}") for oc in range(CC)]
                kt = [qk_pool.tile([128, HW], BF16, tag=f"kt{oc}", name=f"kt{oc}") for oc in range(CC)]
                v_sb = v_pool.tile([128, 32 * C], BF16, tag="v", name="v")
                for g in range(8):
                    t8 = g
                    # Q/K projections for column block t8
                    for wname, bias_sb, dest in (("wq", bq_sb, qt), ("wk", bk_sb, kt)):
                        for oc in range(CC):
                            ps = pp_pool.tile([128, 512], F32, tag="ps", name="ps")
                            for cc in range(CC):
                                nc.tensor.matmul(
                                    ps[:],
                                    w_sb[wname][:, cc * C + oc * 128: cc * C + oc * 128 + 128],
                                    xb[cc][:, t8 * 512:(t8 + 1) * 512],
                                    start=(cc == 0),
                                    stop=(cc == 1),
                                )
                            nc.scalar.activation(
                                dest[oc][:, t8 * 512:(t8 + 1) * 512],
                                ps[:],
                                AF.Identity,
                                bias=bias_sb[:, oc:oc + 1],
                            )
                    # V projection for hw chunks 4g..4g+3 (jj = 2g, 2g+1)
                    # v_sb[p, j*256 + c] = V[j*128 + p, c]
                    for jj in (2 * g, 2 * g + 1):
                        ps = pp_pool.tile([128, 512], F32, tag="ps", name="ps")
                        for u in range(2):
                            j = 2 * jj + u
                            for cc in range(CC):
                                nc.tensor.matmul(
                                    ps[:, u * C:(u + 1) * C],
                                    xb[cc][:, j * 128:(j + 1) * 128],
                                    w_sb["wv"][:, cc * C:(cc + 1) * C],
                                    start=(cc == 0),
                                    stop=(cc == 1),
                                )
                        nc.vector.tensor_add(
                            v_sb[:, jj * 512:(jj + 1) * 512], ps[:], bvb_sb[:]
                        )

                    # --- attention for group g ---
                    # j indexes h-pairs (h0=2j, h1=2j+1); qt/kt cols
                    # j*128 .. (j+1)*128.  attnT per pair is built
                    # block-diagonal ([128, 128]: h0 in [0:64, 0:64], h1 in
                    # [64:128, 64:128], zeros elsewhere) so the attout matmul
                    # contracts over the full 128 partitions in one shot
                    # (row-offset matmuls are broken in this stack).
                    # attnT for 4 j's accumulates into one PSUM tile
                    # (j at cols jj4*64; h0 rows 0:64, h1 rows 64:128)
                    atps = atp_pool.tile([128, 256], F32, tag="atps", name="atps")
                    for jj4 in range(4):
                        j = 4 * g + jj4
                        for half in range(2):  # h0 / h1 within the j block
                            p0 = half * 64
                            col = j * 128 + half * 64
                            for cc in range(CC):
                                nc.tensor.matmul(
                                    atps[p0:p0 + 64, jj4 * 64:(jj4 + 1) * 64],
                                    kt[cc][:, col:col + 64],
                                    qt[cc][:, col:col + 64],
                                    start=(cc == 0),
                                    stop=(cc == 1),
                                )
                    # at4 holds 4 block-diagonal [128, 128] attnT matrices
                    at4 = at_pool.tile([128, 512], BF16, tag="at", name="at")
                    at4r = at4.rearrange("p (j n) -> p j n", j=4)
                    atpsr = atps.rearrange("p (j n) -> p j n", j=4)
                    nc.gpsimd.memset(at4r[0:64, :, 64:128], 0.0)
                    nc.gpsimd.memset(at4r[64:128, :, 0:64], 0.0)
                    nc.scalar.activation(at4r[0:64, :, 0:64], atpsr[0:64, :, :],
                                         AF.Copy, bias=0.0)
                    nc.scalar.activation(at4r[64:128, :, 64:128], atpsr[64:128, :, :],
                                         AF.Copy, bias=0.0)

                    aop = [aop_pool.tile([128, 512], F32, tag="aop", name="aop") for _ in range(CC)]
                    for jj4 in range(4):
                        j = 4 * g + jj4
                        for cc in range(CC):
                            nc.tensor.matmul(
                                aop[cc][:, jj4 * 128:(jj4 + 1) * 128],
                                v_sb[:, j * C + cc * 128: j * C + (cc + 1) * 128],
                                at4[:, jj4 * 128:(jj4 + 1) * 128],
                                start=True,
                                stop=True,
                            )
                    for cc in range(CC):
                        osb = out_pool.tile([128, 512], F32, tag="osb", name="osb")
                        nc.vector.tensor_add(
                            osb[:], aop[cc][:], xb[cc][:, g * 512:(g + 1) * 512]
                        )
                        nc.sync.dma_start(
                            out_d[s, cc * 128:(cc + 1) * 128, g * 512:(g + 1) * 512],
                            osb[:],
                        )
    nc.compile()
    return nc


_NC_CACHE = None


def _get_program():
    global _NC_CACHE
    if _NC_CACHE is None:
        _NC_CACHE = build_program()
    return _NC_CACHE


def _make_in_maps(a, wq, bq, wk, bk, wv, bv):
    bf = ml_dtypes.bfloat16

    def pack_w(w):
        # w [c_out, c_in] -> SBUF view [128, cc*256 + c_out]
        w_t = np.asarray(w, np.float32).T.astype(bf)          # [c_in, c_out]
        return np.ascontiguousarray(
            w_t.reshape(2, 128, C).transpose(1, 0, 2).reshape(128, 2 * C))

    w_all = np.concatenate([pack_w(wq), pack_w(wk), pack_w(wv)], axis=1)
    bq_f = np.asarray(bq, np.float32)
    bk_f = np.asarray(bk, np.float32)
    bqk = np.ascontiguousarray(
        np.stack([bq_f[:128], bq_f[128:], bk_f[:128], bk_f[128:]], axis=1))
    bvb = np.tile(np.asarray(bv, np.float32).reshape(1, C), (128, 2))
    a4 = np.asarray(a, np.float32).reshape(B, C, HW).astype(bf)
    in_maps = []
    for i in range(N_CORES):
        in_maps.append({
            "a_bf": np.ascontiguousarray(a4[i * S:(i + 1) * S]),
            "w_all": w_all, "bqk": bqk, "bvb": bvb,
        })
    return in_maps


def run(a, wq, bq, wk, bk, wv, bv, trace=False, **trace_kw):
    nc = _get_program()
    in_maps = _make_in_maps(a, wq, bq, wk, bk, wv, bv)
    res = run_bass_kernel_spmd(nc, in_maps, list(range(N_CORES)), trace=trace, **trace_kw)
    out = np.concatenate([np.asarray(r["out"]) for r in res.results], axis=0)
    return out.reshape(B, C, H, W), res


def kernel(a, wq, bq, wk, bk, wv, bv):
    out, _ = run(a, wq, bq, wk, bk, wv, bv, trace=False)
    return out



# revision 2
# speedup vs baseline: 214.7546x; 214.7546x over previous
"""Trainium2 Bass kernel for nn_MultiHeadAttn (unnormalized spatial attention).

Reference computation (per sample s of B=16):
    X = a[s]               # [C=256, HW=4096]  (H=64 rows of W=64)
    QT = wq @ X + bq       # [C, HW]   (q channels on rows)
    KT = wk @ X + bk
    V  = (wv @ X + bv).T   # [HW, C]   (hw on rows)
    per h: attnT_h = K_h @ Q_h^T        # [W, W]  == (Q_h K_h^T)^T
           attoutT_h = V_h^T @ attnT_h  # [C, W]
    out[s] = a[s] + attoutT (reassembled [C, HW])

Sharding: data-parallel over batch, 2 samples per core on 8 cores.
All matmuls in bf16 (fp32 PSUM accumulation); residual in fp32 PSUM +
bf16 operand, stored as bf16 and widened to f32 on the host.

Schedule: 16 global groups (2 samples x 8 column blocks of 512 hw
positions) run through a 2-stage software pipeline so the PE never
waits on the ACT engine:
    iteration i: projections(i) | QK^T(i-1) + extract | attout(i-2)
Input loads ride the ACT hardware DGE queue, stores the SP queue, so
sample 1's loads never sit behind sample 0's output stores.
"""

import numpy as np
import ml_dtypes

import concourse.bass as bass
import concourse.mybir as mybir
import concourse.tile as tile
from concourse import bacc
from concourse.bass_utils import run_bass_kernel_spmd

BF16 = mybir.dt.bfloat16
F32 = mybir.dt.float32
AF = mybir.ActivationFunctionType

N_CORES = 8
B, C, H, W = 16, 256, 64, 64
HW = H * W               # 4096
S = B // N_CORES         # samples per core = 2
CC = C // 128            # channel chunks = 2
G = 8                    # column blocks per sample (512 hw each)
NG = S * G               # global groups per core = 16
LOOKAHEAD = 3            # input-load groups ahead of compute


def build_program():
    nc = bacc.Bacc("TRN2", target_bir_lowering=False, debug=False)

    # block-major input: a_blk[s, cc, g, p, col] = a[s, cc*128+p, g*512+col]
    # so each (s, cc, g) slab is one contiguous 128 KiB DMA.
    a_in = nc.dram_tensor("a_blk", [S, CC, G, 128, 512], BF16, kind="ExternalInput")
    # packed constants (see _make_in_maps): weights [128, 3*512] bf16 with
    # w_all[p, w*512 + cc*256 + o] = w^T[cc*128 + p, o]; biases [128, 4] f32
    # as columns (bq0, bq1, bk0, bk1); bvb [128, 512] f32 = bv tiled twice.
    w_in = nc.dram_tensor("w_all", [128, 3 * 512], BF16, kind="ExternalInput")
    bqk_in = nc.dram_tensor("bqk", [128, 4], F32, kind="ExternalInput")
    bvb_in = nc.dram_tensor("bvb", [128, 512], F32, kind="ExternalInput")
    # block-major bf16 output, same layout as a_blk
    out_d = nc.dram_tensor("out", [S, CC, G, 128, 512], BF16, kind="ExternalOutput")

    with tile.TileContext(nc) as tc:
        with (
            tc.tile_pool(name="const", bufs=1) as const_pool,
            tc.tile_pool(name="xb", bufs=2 * CC) as xb_pool,
            tc.tile_pool(name="qk", bufs=2) as qk_pool,
            tc.tile_pool(name="vsb", bufs=2) as v_pool,
            tc.tile_pool(name="osb", bufs=8) as out_pool,
            tc.tile_pool(name="qkps", bufs=3, space=bass.MemorySpace.PSUM) as qkps_pool,
            tc.tile_pool(name="vps", bufs=2, space=bass.MemorySpace.PSUM) as vps_pool,
            tc.tile_pool(name="atp", bufs=1, space=bass.MemorySpace.PSUM) as atp_pool,
            tc.tile_pool(name="aop", bufs=2, space=bass.MemorySpace.PSUM) as aop_pool,
        ):
            # --- persistent constants: weights on the SP queue (stores only
            # start two iterations later, so no FIFO conflict); wq first so
            # the first projection can start as early as possible.
            w_all_sb = const_pool.tile([128, 3 * 512], BF16, tag="w_all")
            bqk_sb = const_pool.tile([128, 4], F32, tag="bqk")
            bvb_sb = const_pool.tile([128, 512], F32, tag="bvb")
            nc.sync.dma_start(w_all_sb[:, 0:512], w_in[:, 0:512])
            nc.sync.dma_start(bqk_sb[:], bqk_in[:])
            nc.sync.dma_start(w_all_sb[:, 1024:1536], w_in[:, 1024:1536])
            nc.sync.dma_start(w_all_sb[:, 512:1024], w_in[:, 512:1024])
            nc.sync.dma_start(bvb_sb[:], bvb_in[:])
            w_sb = {"wq": w_all_sb[:, 0:512],
                    "wk": w_all_sb[:, 512:1024],
                    "wv": w_all_sb[:, 1024:1536]}
            bq_sb = bqk_sb[:, 0:2]
            bk_sb = bqk_sb[:, 2:4]

            # at4 tiles hold 4 block-diagonal [128, 128] attnT matrices
            # ([128, 128]: h0 in [0:64, 0:64], h1 in [64:128, 64:128]).
            # The off-diagonal zeros are written ONCE here; extracts only
            # ever write the diagonal blocks, so the zeros persist across
            # reuse and no per-group memset is needed.
            at4_tiles = []
            for t in range(4):
                at4 = const_pool.tile([128, 512], BF16, tag=f"at4_{t}")
                at4r = at4.rearrange("p (j n) -> p j n", j=4)
                nc.gpsimd.memset(at4r[0:64, :, 64:128], 0.0)
                nc.gpsimd.memset(at4r[64:128, :, 0:64], 0.0)
                at4_tiles.append(at4)

            # --- per-sample tiles, allocated lazily ---
            xb_all = {}    # sv -> [xb_cc0, xb_cc1]  ([128, HW] bf16)
            qt_all = {}    # sv -> [qt0, qt1]
            kt_all = {}
            v_all = {}     # sv -> v_sb [128, 32*C]  (V in [hw-part, c] layout)

            def ensure_xb(sv):
                if sv not in xb_all:
                    xb_all[sv] = [xb_pool.tile([128, HW], BF16, tag="xb",
                                               name="xb")
                                  for _ in range(CC)]
                return xb_all[sv]

            def emit_load(gi):
                # input loads on the ACT hardware DGE queue
                sv, g = divmod(gi, G)
                xb = ensure_xb(sv)
                for cc in range(CC):
                    nc.scalar.dma_start(
                        xb[cc][:, g * 512:(g + 1) * 512],
                        a_in[sv, cc, g])

            for gi in range(LOOKAHEAD):
                emit_load(gi)

            # --- pipelined main loop ---
            for i in range(NG + 2):
                # Stage A: projections for group i
                if i < NG:
                    sv, g = divmod(i, G)
                    if i + LOOKAHEAD < NG:
                        emit_load(i + LOOKAHEAD)
                    xb = ensure_xb(sv)
                    if g == 0:
                        qt_all[sv] = [qk_pool.tile([128, HW], BF16,
                                                   tag=f"qt{oc}", name=f"qt{oc}")
                                      for oc in range(CC)]
                        kt_all[sv] = [qk_pool.tile([128, HW], BF16,
                                                   tag=f"kt{oc}", name=f"kt{oc}")
                                      for oc in range(CC)]
                        v_all[sv] = v_pool.tile([128, 32 * C], BF16,
                                                tag="v", name="v")
                    qt, kt, v_sb = qt_all[sv], kt_all[sv], v_all[sv]

                    # Q/K projections for column block g
                    for wname, bias_sb, dest in (("wq", bq_sb, qt),
                                                 ("wk", bk_sb, kt)):
                        for oc in range(CC):
                            ps = qkps_pool.tile([128, 512], F32, tag="ps",
                                                name="ps")
                            for cc in range(CC):
                                nc.tensor.matmul(
                                    ps[:],
                                    w_sb[wname][:, cc * C + oc * 128:
                                                cc * C + oc * 128 + 128],
                                    xb[cc][:, g * 512:(g + 1) * 512],
                                    start=(cc == 0),
                                    stop=(cc == 1),
                                )
                            nc.scalar.activation(
                                dest[oc][:, g * 512:(g + 1) * 512],
                                ps[:],
                                AF.Identity,
                                bias=bias_sb[:, oc:oc + 1],
                            )
                    # V projection for hw chunks 4g..4g+3 (jj = 2g, 2g+1)
                    # v_sb[p, j*256 + c] = V[j*128 + p, c]
                    for jj in (2 * g, 2 * g + 1):
                        ps = vps_pool.tile([128, 512], F32, tag="vps",
                                           name="vps")
                        for u in range(2):
                            j = 2 * jj + u
                            for cc in range(CC):
                                nc.tensor.matmul(
                                    ps[:, u * C:(u + 1) * C],
                                    xb[cc][:, j * 128:(j + 1) * 128],
                                    w_sb["wv"][:, cc * C:(cc + 1) * C],
                                    start=(cc == 0),
                                    stop=(cc == 1),
                                )
                        nc.vector.tensor_add(
                            v_sb[:, jj * 512:(jj + 1) * 512], ps[:], bvb_sb[:]
                        )

                # Stage B: attnT for group i-1
                ai = i - 1
                if 0 <= ai < NG:
                    sv, g = divmod(ai, G)
                    qt, kt = qt_all[sv], kt_all[sv]
                    # attnT per h-pair j is built block-diagonal so the
                    # attout matmul contracts over the full 128 partitions
                    # (row-offset matmuls are broken in this stack).
                    # attnT for 4 j's accumulates into one PSUM tile
                    # (j at cols jj4*64; h0 rows 0:64, h1 rows 64:128).
                    atps = atp_pool.tile([128, 256], F32, tag="atps",
                                         name="atps")
                    for jj4 in range(4):
                        j = 4 * g + jj4
                        for half in range(2):  # h0 / h1 within the j block
                            p0 = half * 64
                            col = j * 128 + half * 64
                            for cc in range(CC):
                                nc.tensor.matmul(
                                    atps[p0:p0 + 64, jj4 * 64:(jj4 + 1) * 64],
                                    kt[cc][:, col:col + 64],
                                    qt[cc][:, col:col + 64],
                                    start=(cc == 0),
                                    stop=(cc == 1),
                                )
                    at4 = at4_tiles[ai % 4]
                    at4r = at4.rearrange("p (j n) -> p j n", j=4)
                    atpsr = atps.rearrange("p (j n) -> p j n", j=4)
                    nc.scalar.activation(at4r[0:64, :, 0:64],
                                         atpsr[0:64, :, :], AF.Copy, bias=0.0)
                    nc.scalar.activation(at4r[64:128, :, 64:128],
                                         atpsr[64:128, :, :], AF.Copy, bias=0.0)

                # Stage C: attout + residual + store for group i-2
                oi = i - 2
                if 0 <= oi:
                    sv, g = divmod(oi, G)
                    xb, v_sb = xb_all[sv], v_all[sv]
                    at4 = at4_tiles[oi % 4]
                    aop = [aop_pool.tile([128, 512], F32, tag="aop",
                                         name="aop") for _ in range(CC)]
                    for jj4 in range(4):
                        j = 4 * g + jj4
                        for cc in range(CC):
                            nc.tensor.matmul(
                                aop[cc][:, jj4 * 128:(jj4 + 1) * 128],
                                v_sb[:, j * C + cc * 128:
                                     j * C + (cc + 1) * 128],
                                at4[:, jj4 * 128:(jj4 + 1) * 128],
                                start=True,
                                stop=True,
                            )
                    for cc in range(CC):
                        osb = out_pool.tile([128, 512], BF16, tag="osb",
                                            name="osb")
                        nc.vector.tensor_add(
                            osb[:], aop[cc][:], xb[cc][:, g * 512:(g + 1) * 512]
                        )
                        nc.sync.dma_start(out_d[sv, cc, g], osb[:])
    nc.compile()
    return nc


_NC_CACHE = None


def _get_program():
    global _NC_CACHE
    if _NC_CACHE is None:
        _NC_CACHE = build_program()
    return _NC_CACHE


def _make_in_maps(a, wq, bq, wk, bk, wv, bv):
    bf = ml_dtypes.bfloat16

    def pack_w(w):
        # w [c_out, c_in] -> SBUF view [128, cc*256 + c_out]
        w_t = np.asarray(w, np.float32).T.astype(bf)          # [c_in, c_out]
        return np.ascontiguousarray(
            w_t.reshape(2, 128, C).transpose(1, 0, 2).reshape(128, 2 * C))

    w_all = np.concatenate([pack_w(wq), pack_w(wk), pack_w(wv)], axis=1)
    bq_f = np.asarray(bq, np.float32)
    bk_f = np.asarray(bk, np.float32)
    bqk = np.ascontiguousarray(
        np.stack([bq_f[:128], bq_f[128:], bk_f[:128], bk_f[128:]], axis=1))
    bvb = np.tile(np.asarray(bv, np.float32).reshape(1, C), (128, 2))
    # block-major bf16 input: [B, CC, G, 128, 512]
    a_bf = np.asarray(a, np.float32).reshape(B, CC, 128, G, 512).astype(bf)
    a_blk = a_bf.transpose(0, 1, 3, 2, 4)
    in_maps = []
    for i in range(N_CORES):
        in_maps.append({
            "a_blk": np.ascontiguousarray(a_blk[i * S:(i + 1) * S]),
            "w_all": w_all, "bqk": bqk, "bvb": bvb,
        })
    return in_maps


def run(a, wq, bq, wk, bk, wv, bv, trace=False, **trace_kw):
    nc = _get_program()
    in_maps = _make_in_maps(a, wq, bq, wk, bk, wv, bv)
    res = run_bass_kernel_spmd(nc, in_maps, list(range(N_CORES)), trace=trace,
                               **trace_kw)
    out = np.concatenate([np.asarray(r["out"]) for r in res.results], axis=0)
    # [B, CC, G, 128, 512] bf16 -> [B, C, H, W] f32
    out = out.transpose(0, 1, 3, 2, 4).astype(np.float32)
    return np.ascontiguousarray(out.reshape(B, C, H, W)), res


def kernel(a, wq, bq, wk, bk, wv, bv):
    out, _ = run(a, wq, bq, wk, bk, wv, bv, trace=False)
    return out
